# revision 4
# baseline (speedup 1.0000x reference)
"""CostVolumeLayer Trainium2 kernel.

Computes the local cost volume: for search_range R=4,
  out[b, di*9+dj, i, j] = sum_c src[b,c,i,j] * tgt_zp[b,c,i-2R+di, j-2R+dj]
(tgt zero-padded outside its bounds; the window is OFF-CENTER, covering
tgt rows i-8..i and cols j-8..j — faithful to the torch reference, whose
window indices index the zero-padded tensor directly and whose negative
indices wrap into the zero pad).

Strategy (8 NeuronCores, SPMD):
  - Shard: core c -> batch b = c//2, row-half r0 = 32*(c%2). Each core gets
    src shard [C=128, 32, 128] and a zero-padded tgt halo shard
    [C=128, 40, 136] (host pre-pads; halo = R rows/cols each side), bf16.
  - Device: for each 16x8 pixel block of the shard, one TensorE matmul
    lhsT = src block [K=C=128, M=128 pixels], rhs = tgt window
    [K=128, N=24x16=384] -> PSUM Gram [128, 384]. Two blocks share a
    2-bank PSUM tile; one strided PSUM->SBUF fp16 copy drains the pair
    (engines alternated DVE/ACT). Banded strided DMAs dump the stage.
  - Band: pixel partition p = mi*8+mj only needs Gram cols
    (mi+di)*16+(mj+dj), so partition quarter q (mi in [4q, 4q+3]) keeps
    cols 64q..64q+191: 4 quarter-DMAs per group read a 192-wide band.
  - Queues: input DMAs issue from the Scalar engine (qActDynamicHW) and
    output DMAs from Sync (qSPDynamicHW) so input and output do not
    serialize on one FIFO queue; chunks drain in need-order.
  - Host: zero-FLOP banded-diagonal gather from the Gram blocks into the
    [B, 81, H, W] output (the per-pixel diagonal is a per-partition-skewed
    pattern that engine/DMA access patterns cannot express on-chip).
"""

import numpy as np

R = 4
D = 2 * R + 1          # 9
B, C, H, W = 4, 128, 64, 128
NCORES = 8
HS = H // 2            # 32 rows per core shard
TH = HS + 2 * R        # 40 padded tgt rows per shard
TW = W + 2 * R         # 136 padded tgt cols
BI, BJ = 16, 8         # pixel block: 16 rows x 8 cols = 128 = M
NBI, NBJ = HS // BI, W // BJ   # 2 x 16 = 32 blocks per core
WIN_I, WIN_J = BI + 2 * R, BJ + 2 * R  # 24 x 16 window
NW = WIN_I * WIN_J     # 384 streamed columns per block
NBLK = NBI * NBJ
GRP = 16               # blocks per output DMA group (= one block-row)
NGRP = NBLK // GRP     # 2 groups; 4 banded quarter-DMAs each
BANDW = 192            # band width per partition quarter
QSTEP = 4 * WIN_J      # 64: band column offset step per quarter

_compiled = None


def _build_bass():
    import concourse.mybir as mybir
    from concourse import bacc
    from concourse.tile import TileContext

    f32 = mybir.dt.float32
    in_dt = mybir.dt.bfloat16
    dump_dt = mybir.dt.float16
    nc = bacc.Bacc()
    # single combined input: [C, HS*W (block-reordered src) ++ TH*TW (padded
    # tgt)] in bf16.
    E = HS * W + TH * TW
    inp = nc.dram_tensor("inp", [C, E], in_dt, kind="ExternalInput")
    gout = nc.dram_tensor("gout", [NGRP, 128, GRP * BANDW], dump_dt,
                          kind="ExternalOutput")
    gout_ap = gout.ap()

    with TileContext(nc) as tc:
        with (
            tc.tile_pool(name="inp", bufs=1) as inp_pool,
            tc.tile_pool(name="g", bufs=NGRP) as gpool,
            tc.tile_pool(name="psum", bufs=3, space="PSUM") as psum_pool,
            tc.tile_pool(name="warmpsum", bufs=1, space="PSUM") as warm_pool,
        ):
            # src arrives block-reordered from the host: [C, blk, 128 pixels]
            # so each block's weights are one contiguous free dim.
            a = inp_pool.tile([C, E], in_dt)

            def s_view():
                return a[:, :HS * W]

            def t_view():
                return a[:, HS * W:].rearrange("c (i j) -> c i j", j=TW)

            # PE warm-up: dummy matmuls during the input-DMA wait keep the
            # HAM clock gate busy so it flips to 8/8 before the real matmuls.
            warm = inp_pool.tile([128, 128], in_dt)
            nc.vector.memset(warm, 0.0)
            wps = warm_pool.tile([1, 128], f32)
            for _ in range(32):
                nc.tensor.matmul(wps, warm[:, :1], warm, start=True, stop=True)

            # Chunked input load on the Scalar HWDGE queue (separate FIFO
            # from the Sync queue carrying output DMAs). Chunks on one queue
            # drain strictly in issue order at full queue bandwidth, so
            # need-order issue = need-order arrival; no dep chaining.
            SRCC = 8 * 128              # src chunk: 8 blocks = 1024 elems
            TGTC = 8 * TW               # tgt chunk: 8 rows
            def src_chunk(i):
                return nc.scalar.dma_start(
                    out=a[:, i * SRCC:(i + 1) * SRCC],
                    in_=inp.ap()[:, i * SRCC:(i + 1) * SRCC])
            def tgt_chunk(i):
                o = HS * W + i * TGTC
                return nc.scalar.dma_start(out=a[:, o:o + TGTC],
                                           in_=inp.ap()[:, o:o + TGTC])
            # block-row 0 (blocks 0-15) needs src chunks 0-1 + tgt rows 0-23;
            # block-row 1 needs src 2-3 + tgt rows 16-39.
            src_chunk(0), tgt_chunk(0), tgt_chunk(1), tgt_chunk(2)
            src_chunk(1), src_chunk(2), tgt_chunk(3), tgt_chunk(4)
            src_chunk(3)

            for grp in range(NGRP):
                stage = gpool.tile([128, GRP * NW], dump_dt)
                for pair in range(GRP // 2):
                    # two blocks share one 2-bank PSUM tile; matmul outputs
                    # at 512-elem offsets so each stays inside one bank
                    ps = psum_pool.tile([128, 1024], f32)
                    psv = ps.rearrange("p (k w) -> p k w", w=512)
                    for k in range(2):
                        blk = grp * GRP + pair * 2 + k
                        bi, bj = divmod(blk, NBJ)
                        lhsT = s_view()[:, blk * 128:(blk + 1) * 128]
                        rhs = t_view()[:, bi * BI: bi * BI + WIN_I,
                                       bj * BJ: bj * BJ + WIN_J]
                        nc.tensor.matmul(psv[:, k, 0:NW], lhsT, rhs,
                                         start=True, stop=True)
                    # one strided copy drains both blocks (cheaper than two:
                    # DVE/ACT cost is fixed overhead + free-elems); alternate
                    # engines so adjacent pairs copy in parallel
                    dst = (stage[:, pair * 2 * NW:(pair + 1) * 2 * NW]
                           .rearrange("p (k w) -> p k w", w=NW))
                    src = psv[:, :, 0:NW]
                    if pair % 2 == 0:
                        nc.vector.tensor_copy(dst, src)
                    else:
                        nc.scalar.copy(dst, src)
                # banded output DMAs: partition quarter q keeps cols
                # 64q..64q+191 of each block's 384 Gram columns.
                sv = stage.rearrange("p (k w) -> p k w", w=NW)
                gv = gout_ap[grp].rearrange("p (k w) -> p k w", w=BANDW)
                for q in range(4):
                    nc.sync.dma_start(
                        out=gv[32 * q:32 * q + 32],
                        in_=sv[32 * q:32 * q + 32, :,
                               QSTEP * q:QSTEP * q + BANDW])
    nc.finalize()
    return nc


def _get_compiled():
    global _compiled
    if _compiled is None:
        _compiled = _build_bass()
    return _compiled


def _shard_inputs(src, tgt):
    """Build per-core input maps (host-side shard + zero-pad + bf16)."""
    import ml_dtypes

    bf16 = ml_dtypes.bfloat16
    in_maps = []
    for c in range(NCORES):
        b = c // 2
        r0 = HS * (c % 2)
        # block-reorder: [C, NBI, BI, NBJ, BJ] -> [C, (NBI NBJ), (BI BJ)]
        s = np.ascontiguousarray(
            src[b, :, r0:r0 + HS, :]
            .reshape(C, NBI, BI, NBJ, BJ)
            .transpose(0, 1, 3, 2, 4)
        ).reshape(C, HS * W)
        tp = np.zeros((C, TH, TW), dtype=np.float32)
        # The reference's window for output pixel (i, j) covers tgt rows
        # i-2R..i and cols j-2R..j (off-center, faithful to the torch quirk:
        # window indices index the PADDED tensor directly, so padded index
        # i-R+di = tgt row i-2R+di). Device pairs src local row il with
        # shard-padded row il+di, so shard row q holds tgt row r0+q-2R;
        # shard col x holds tgt col x-2R.
        lo = r0 - 2 * R
        hi = r0 + HS
        clo = max(lo, 0)
        tp[:, clo - lo: clo - lo + (hi - clo), 2 * R: 2 * R + W] = tgt[b, :, clo:hi, :]
        inp = np.concatenate([s, tp.reshape(C, TH * TW)], axis=1)
        in_maps.append({"inp": np.ascontiguousarray(inp.astype(bf16))})
    return in_maps


# host-side gather indices: out[k=(di,dj)] at pixel (mi,mj) of a block sits at
# Gram column n = (mi+di)*WIN_J + (mj+dj); the device band dump stores
# columns shifted by QSTEP*(mi//4) for partition quarter mi//4.
_mi = np.arange(BI)[:, None, None, None]
_mj = np.arange(BJ)[None, :, None, None]
_di = np.arange(D)[None, None, :, None]
_dj = np.arange(D)[None, None, None, :]
_NIDX = ((_mi + _di) * WIN_J + (_mj + _dj)
         - QSTEP * (_mi // 4)).reshape(BI, BJ, D * D)  # [16,8,81]


def _unshard_output(results):
    out = np.empty((B, D * D, H, W), dtype=np.float32)
    for c in range(NCORES):
        b = c // 2
        r0 = HS * (c % 2)
        g = (results[c]["gout"]
             .astype(np.float32)
             .reshape(NGRP, 128, GRP, BANDW)
             .transpose(0, 2, 1, 3)
             .reshape(NBI, NBJ, BI, BJ, BANDW))
        # gather: v[bi,bj,mi,mj,k] = g[bi,bj,mi,mj,_NIDX[mi,mj,k]]
        v = np.take_along_axis(g, _NIDX[None, None], axis=-1)
        # -> out[b, k, r0+bi*BI+mi, bj*BJ+mj]
        v = v.transpose(4, 0, 2, 1, 3)  # [81, NBI, BI, NBJ, BJ]
        out[b, :, r0:r0 + HS, :] = v.reshape(D * D, HS, W)
    return out


def kernel(src, tgt):
    from concourse.bass_utils import run_bass_kernel_spmd

    src = np.asarray(src, dtype=np.float32)
    tgt = np.asarray(tgt, dtype=np.float32)
    nc = _get_compiled()
    in_maps = _shard_inputs(src, tgt)
    res = run_bass_kernel_spmd(nc, in_maps, core_ids=list(range(NCORES)))
    return _unshard_output(res.results)
